# revision 5
# baseline (speedup 1.0000x reference)
"""GAT-style message-passing kernel for Trainium2 (8 NeuronCores, batch-parallel).

Math (per batch b):
    alpha[n]  = K[b,n,:] @ wk          (+ const(b): dropped — softmax shift-invariant)
    e[n]      = exp(alpha[n]) * adj[b,n]
    attn[n]   = e[n] / sum(e)
    w0 = attn * s_mask ; w1 = attn * (1 - s_mask)
    attn_sum[b] = (w0 @ V[b]) @ Wr0.T + (w1 @ V[b]) @ Wr1.T + Q[b] @ Wri.T

The V-aggregation is done BEFORE the HxH projections (exact reordering), so the
kernel is memory-bound on streaming K and V exactly once per element.
"""

import numpy as np
from contextlib import ExitStack

import concourse.bass as bass
import concourse.tile as tile
from concourse import bacc, mybir
from concourse.bass_utils import run_bass_kernel_spmd
from concourse.masks import make_identity

B, N, H = 32, 2048, 512
NCORES = 8
BPC = B // NCORES        # batches per core
P = 128                  # partitions
NT = N // P              # 16 n-tiles per batch
CH = 8                   # n-tiles per DMA chunk (2 MB)
NCH = NT // CH           # chunks per batch
HC = H // P              # h-chunks

f32 = mybir.dt.float32
i32 = mybir.dt.int32
FT = mybir.ActivationFunctionType
ALU = mybir.AluOpType
AX = mybir.AxisListType


def _build():
    nc = bacc.Bacc(
        "TRN2", target_bir_lowering=False, debug=False, num_devices=NCORES
    )
    Kd = nc.dram_tensor("K", [BPC, N, H], f32, kind="ExternalInput").ap()
    Vd = nc.dram_tensor("V", [BPC, N, H], f32, kind="ExternalInput").ap()
    adjd = nc.dram_tensor("adj", [BPC, N], i32, kind="ExternalInput").ap()
    smd = nc.dram_tensor("s_mask", [BPC, N], i32, kind="ExternalInput").ap()
    Qd = nc.dram_tensor("Q", [BPC, H], f32, kind="ExternalInput").ap()
    wkd = nc.dram_tensor("wk", [1, H], f32, kind="ExternalInput").ap()
    Wds = [
        nc.dram_tensor(nm, [H, H], f32, kind="ExternalInput").ap()
        for nm in ("Wr0", "Wr1", "Wri")
    ]
    attn_out = nc.dram_tensor("attn_out", [BPC, N], f32, kind="ExternalOutput").ap()
    asum_out = nc.dram_tensor("asum_out", [BPC, H], f32, kind="ExternalOutput").ap()

    with tile.TileContext(nc) as tc, ExitStack() as ctx:
        const = ctx.enter_context(tc.tile_pool(name="const", bufs=1))
        kpool = ctx.enter_context(tc.tile_pool(name="kpool", bufs=3))
        vpool = ctx.enter_context(tc.tile_pool(name="vpool", bufs=3))
        wstage = ctx.enter_context(tc.tile_pool(name="wstage", bufs=2))
        scratch = ctx.enter_context(tc.tile_pool(name="scratch", bufs=2))
        scratch2 = ctx.enter_context(tc.tile_pool(name="scratch2", bufs=2))
        small = ctx.enter_context(tc.tile_pool(name="small", bufs=1))
        psum = ctx.enter_context(tc.tile_pool(name="psum", bufs=2, space="PSUM"))
        aggp = ctx.enter_context(tc.tile_pool(name="aggp", bufs=1, space="PSUM"))
        outp = ctx.enter_context(tc.tile_pool(name="outp", bufs=1, space="PSUM"))
        qpsp = ctx.enter_context(tc.tile_pool(name="qpsp", bufs=1, space="PSUM"))

        ident = const.tile([P, P], f32)
        make_identity(nc, ident)
        ones = const.tile([P, 1], f32)
        nc.gpsimd.memset(ones, 1.0)

        # wk replicated to all 128 partitions (one small broadcast DMA)
        wkb = const.tile([P, H], f32)
        nc.gpsimd.dma_start(out=wkb, in_=wkd.to_broadcast((P, H)))

        # Load the three HxH weights and transpose on-chip via PE:
        # WT[wi][:, c, :] is W^T[c*128:(c+1)*128, :]  (contraction dim h on partitions)
        WTs = []
        for wi, wd in enumerate(Wds):
            wt = const.tile([P, HC, H], f32, tag=f"WT{wi}")
            WTs.append(wt)
            for t in range(HC):
                st = wstage.tile([P, H], f32)
                nc.sync.dma_start(out=st, in_=wd[t * P : (t + 1) * P, :])
                for c in range(HC):
                    ps = psum.tile([P, P], f32, tag="bigps")
                    nc.tensor.transpose(ps, st[:, c * P : (c + 1) * P], ident)
                    nc.scalar.copy(wt[:, c, t * P : (t + 1) * P], ps)

        # adj / s_mask: load row-major (cast i32->f32 during DMA), PE-transpose
        # into [p, b, j] layout (n = j*128 + p)
        adj4 = small.tile([BPC, N], f32)
        sm4 = small.tile([BPC, N], f32)
        nc.gpsimd.dma_start(out=adj4, in_=adjd)
        nc.gpsimd.dma_start(out=sm4, in_=smd)
        adjT = small.tile([P, BPC, NT], f32)
        smT = small.tile([P, BPC, NT], f32)
        for src4, dstT in ((adj4, adjT), (sm4, smT)):
            for j in range(NT):
                ps = psum.tile([P, BPC], f32, tag="bigps")
                nc.tensor.transpose(
                    ps, src4[:, j * P : (j + 1) * P], ident[:BPC, :BPC]
                )
                nc.scalar.copy(dstT[:, :, j], ps)

        q_sb = small.tile([BPC, H], f32)
        nc.sync.dma_start(out=q_sb, in_=Qd)

        alphaT = small.tile([P, BPC * NT], f32)     # col = b*NT + j
        em = small.tile([P, BPC, NT], f32)          # exp(alpha)
        eT = small.tile([P, BPC, NT], f32)          # exp(alpha) * adj
        wTl = small.tile([P, BPC, NT, 2], f32)      # stationary (w0, w1) pairs
        colsums = small.tile([P, BPC], f32)

        agg_ps = []
        for b in range(BPC):
            # ---- phase A: stream K[b], alpha[n] = K[b,n,:] . wk ----
            for c in range(NCH):
                kt = kpool.tile([P, CH, H], f32)
                nc.sync.dma_start(
                    out=kt,
                    in_=Kd[b, c * CH * P : (c + 1) * CH * P, :].rearrange(
                        "(t p) h -> p t h", p=P
                    ),
                )
                for t in range(CH):
                    j = c * CH + t
                    sc = scratch.tile([P, H], f32)
                    nc.vector.tensor_mul(sc, kt[:, t, :], wkb)
                    sc2 = scratch2.tile([P, H], f32)
                    nc.scalar.activation(
                        out=sc2,
                        in_=sc,
                        func=FT.Copy,
                        accum_out=alphaT[:, b * NT + j : b * NT + j + 1],
                    )

            # ---- phase A2: e = exp(alpha)*adj, unnormalized w0/w1 ----
            nc.scalar.activation(
                out=em[:, b, :],
                in_=alphaT[:, b * NT : (b + 1) * NT],
                func=FT.Exp,
            )
            nc.vector.tensor_mul(eT[:, b, :], em[:, b, :], adjT[:, b, :])
            nc.vector.reduce_sum(out=colsums[:, b : b + 1], in_=eT[:, b, :], axis=AX.X)
            nc.vector.tensor_mul(wTl[:, b, :, 0], eT[:, b, :], smT[:, b, :])
            nc.vector.tensor_sub(wTl[:, b, :, 1], eT[:, b, :], wTl[:, b, :, 0])

            # ---- phase B: stream V[b], aggregate (u0, u1) in PSUM ----
            aps = aggp.tile([2, H], f32, tag=f"agg{b}")
            agg_ps.append(aps)
            for c in range(NCH):
                vt = vpool.tile([P, CH, H], f32)
                nc.sync.dma_start(
                    out=vt,
                    in_=Vd[b, c * CH * P : (c + 1) * CH * P, :].rearrange(
                        "(t p) h -> p t h", p=P
                    ),
                )
                for t in range(CH):
                    j = c * CH + t
                    nc.tensor.matmul(
                        aps,
                        lhsT=wTl[:, b, j, :],
                        rhs=vt[:, t, :],
                        start=(j == 0),
                        stop=(j == NT - 1),
                    )

        # ---- softmax denominators: S[b] = sum_n e  (partition-sum via PE) ----
        S_ps = outp.tile([BPC, 1], f32, tag="outps")
        nc.tensor.matmul(S_ps, lhsT=colsums, rhs=ones, start=True, stop=True)
        S_sb = small.tile([BPC, 1], f32)
        nc.scalar.copy(S_sb, S_ps)
        r = small.tile([BPC, 1], f32)
        nc.vector.reciprocal(r, S_sb)

        # ---- attn output: transpose eT -> [b, n] rows, scale by 1/S ----
        e4 = small.tile([BPC, N], f32)
        for j in range(NT):
            ps = psum.tile([BPC, P], f32, tag="bigps")
            nc.tensor.transpose(ps, eT[:, :, j], ident)
            nc.scalar.copy(e4[:, j * P : (j + 1) * P], ps)
        attn_sb = small.tile([BPC, N], f32)
        nc.scalar.activation(out=attn_sb, in_=e4, func=FT.Copy, scale=r)
        nc.sync.dma_start(out=attn_out, in_=attn_sb)

        # ---- unscaled aggregates to SBUF, transpose (u0,u1)/q to [h, ...] layout ----
        u_pairs = []
        for b in range(BPC):
            up = small.tile([2, H], f32, tag=f"upair{b}")
            u_pairs.append(up)
            nc.scalar.copy(up, agg_ps[b])
        u01T = small.tile([P, HC, BPC, 2], f32)   # [h_in_chunk, c, b, {u0,u1}]
        qT = small.tile([P, HC, BPC], f32)
        for b in range(BPC):
            for c in range(HC):
                ps = psum.tile([P, 2], f32, tag="bigps")
                nc.tensor.transpose(
                    ps, u_pairs[b][:, c * P : (c + 1) * P], ident[:2, :2]
                )
                nc.scalar.copy(u01T[:, c, b, :], ps)
        for c in range(HC):
            ps = psum.tile([P, BPC], f32, tag="bigps")
            nc.tensor.transpose(
                ps, q_sb[:, c * P : (c + 1) * P], ident[:BPC, :BPC]
            )
            nc.scalar.copy(qT[:, c, :], ps)

        # ---- final projections ----
        # P1 = u0_raw @ Wr0^T + u1_raw @ Wr1^T   (scaled by 1/S on PSUM->SBUF copy)
        # P2 = Q @ Wri^T
        p1_ps = outp.tile([BPC, H], f32, tag="outps")
        k_i = 0
        for c in range(HC):
            for ui, wt in ((0, WTs[0]), (1, WTs[1])):
                nc.tensor.matmul(
                    p1_ps,
                    lhsT=u01T[:, c, :, ui],
                    rhs=wt[:, c, :],
                    start=(k_i == 0),
                    stop=(k_i == 2 * HC - 1),
                )
                k_i += 1
        p2_ps = qpsp.tile([BPC, H], f32, tag="qps")
        for c in range(HC):
            nc.tensor.matmul(
                p2_ps,
                lhsT=qT[:, c, :],
                rhs=WTs[2][:, c, :],
                start=(c == 0),
                stop=(c == HC - 1),
            )
        asum1 = small.tile([BPC, H], f32)
        nc.scalar.activation(out=asum1, in_=p1_ps, func=FT.Copy, scale=r)
        asum_sb = small.tile([BPC, H], f32)
        nc.vector.tensor_add(asum_sb, asum1, p2_ps)
        nc.sync.dma_start(out=asum_out, in_=asum_sb)

    nc.compile()
    return nc


_NC = None


def _get_nc():
    global _NC
    if _NC is None:
        _NC = _build()
    return _NC


def run(inputs, trace=False, tmpdir=None):
    """Run on 8 cores; returns ((attn, attn_sum), BassKernelResults)."""
    nc = _get_nc()
    Q = np.asarray(inputs["Q"], dtype=np.float32)
    K = np.asarray(inputs["K"], dtype=np.float32)
    V = np.asarray(inputs["V"], dtype=np.float32)
    adj = np.asarray(inputs["adj"], dtype=np.int32)
    s_mask = np.asarray(inputs["s_mask"], dtype=np.int32)
    w_att = np.asarray(inputs["w_att"], dtype=np.float32)
    Wr0 = np.ascontiguousarray(np.asarray(inputs["Wr0"], dtype=np.float32))
    Wr1 = np.ascontiguousarray(np.asarray(inputs["Wr1"], dtype=np.float32))
    Wri = np.ascontiguousarray(np.asarray(inputs["Wri"], dtype=np.float32))
    wk = np.ascontiguousarray(w_att[:, H:])

    in_maps = []
    for i in range(NCORES):
        sl = slice(i * BPC, (i + 1) * BPC)
        in_maps.append(
            {
                "K": np.ascontiguousarray(K[sl]),
                "V": np.ascontiguousarray(V[sl]),
                "adj": np.ascontiguousarray(adj[sl]),
                "s_mask": np.ascontiguousarray(s_mask[sl]),
                "Q": np.ascontiguousarray(Q[sl]),
                "wk": wk,
                "Wr0": Wr0,
                "Wr1": Wr1,
                "Wri": Wri,
            }
        )
    br = run_bass_kernel_spmd(
        nc, in_maps, list(range(NCORES)), trace=trace, tmpdir=tmpdir
    )
    res = br.results
    attn = np.concatenate([res[i]["attn_out"] for i in range(NCORES)], axis=0)
    asum = np.concatenate([res[i]["asum_out"] for i in range(NCORES)], axis=0)
    return (attn[:, None, :].astype(np.float32), asum.astype(np.float32)), br


def kernel(**inputs):
    (attn, asum), _ = run(inputs, trace=False)
    return attn, asum


# revision 16
# speedup vs baseline: 1.2046x; 1.2046x over previous
"""GAT-style message-passing kernel for Trainium2 (8 NeuronCores, batch-parallel).

Math (per batch b):
    alpha[n]  = K[b,n,:] @ wk          (+ const(b): dropped — softmax shift-invariant)
    e[n]      = exp(alpha[n]) * adj[b,n]
    attn[n]   = e[n] / sum(e)
    w0 = attn * s_mask ; w1 = attn * (1 - s_mask)
    attn_sum[b] = (w0 @ V[b]) @ Wr0.T + (w1 @ V[b]) @ Wr1.T + Q[b] @ Wri.T

The V-aggregation is done BEFORE the HxH projections (exact reordering), so the
kernel is memory-bound on streaming K and V exactly once per element.
"""

import numpy as np
from contextlib import ExitStack

import concourse.bass as bass
import concourse.tile as tile
from concourse import bacc, mybir
from concourse.bass_utils import run_bass_kernel_spmd
from concourse.masks import make_identity

B, N, H = 32, 2048, 512
NCORES = 8
BPC = B // NCORES        # batches per core
P = 128                  # partitions
NT = N // P              # 16 n-tiles per batch
CH = 8                   # n-tiles per DMA chunk (2 MB)
NCH = NT // CH           # chunks per batch
HC = H // P              # h-chunks

f32 = mybir.dt.float32
f32r = mybir.dt.float32r
i32 = mybir.dt.int32
FT = mybir.ActivationFunctionType
ALU = mybir.AluOpType
AX = mybir.AxisListType


def _build():
    nc = bacc.Bacc(
        "TRN2", target_bir_lowering=False, debug=False, num_devices=NCORES
    )
    Kd = nc.dram_tensor("K", [BPC, N, H], f32, kind="ExternalInput").ap()
    Vd = nc.dram_tensor("V", [BPC, N, H], f32r, kind="ExternalInput").ap()
    adjd = nc.dram_tensor("adj", [BPC, N], i32, kind="ExternalInput").ap()
    smd = nc.dram_tensor("s_mask", [BPC, N], i32, kind="ExternalInput").ap()
    Qd = nc.dram_tensor("Q", [BPC, H], f32, kind="ExternalInput").ap()
    wkd = nc.dram_tensor("wk", [1, H], f32, kind="ExternalInput").ap()
    Wds = [
        nc.dram_tensor(nm, [H, H], f32, kind="ExternalInput").ap()
        for nm in ("Wr0", "Wr1", "Wri")
    ]
    attn_out = nc.dram_tensor("attn_out", [BPC, N], f32, kind="ExternalOutput").ap()
    asum_out = nc.dram_tensor("asum_out", [BPC, H], f32, kind="ExternalOutput").ap()

    with tile.TileContext(nc) as tc, ExitStack() as ctx:
        const = ctx.enter_context(tc.tile_pool(name="const", bufs=1))
        kpool = ctx.enter_context(tc.tile_pool(name="kpool", bufs=3))
        vpool = ctx.enter_context(tc.tile_pool(name="vpool", bufs=2))
        wstage = ctx.enter_context(tc.tile_pool(name="wstage", bufs=2))
        scratch = ctx.enter_context(tc.tile_pool(name="scratch", bufs=2))
        scratch2 = ctx.enter_context(tc.tile_pool(name="scratch2", bufs=2))
        scratchg = ctx.enter_context(tc.tile_pool(name="scratchg", bufs=2))
        small = ctx.enter_context(tc.tile_pool(name="small", bufs=1))
        psum = ctx.enter_context(tc.tile_pool(name="psum", bufs=2, space="PSUM"))
        aggp = ctx.enter_context(tc.tile_pool(name="aggp", bufs=1, space="PSUM"))
        outp = ctx.enter_context(tc.tile_pool(name="outp", bufs=1, space="PSUM"))
        qpsp = ctx.enter_context(tc.tile_pool(name="qpsp", bufs=1, space="PSUM"))

        ident = const.tile([P, P], f32)
        make_identity(nc, ident)
        ones = const.tile([P, 1], f32)
        nc.gpsimd.memset(ones, 1.0)

        # wk replicated to all 128 partitions (one small broadcast DMA)
        wkb = const.tile([P, H], f32)
        nc.gpsimd.dma_start(out=wkb, in_=wkd.to_broadcast((P, H)))

        # Load the three HxH weights and transpose on-chip via PE:
        # WT[wi][:, c, :] is W^T[c*128:(c+1)*128, :]  (contraction dim h on partitions)
        WTs = []
        for wi, wd in enumerate(Wds):
            wt = const.tile([P, HC, H], f32r, tag=f"WT{wi}")
            WTs.append(wt)
            for t in range(HC):
                st = wstage.tile([P, H], f32)
                nc.sync.dma_start(out=st, in_=wd[t * P : (t + 1) * P, :])
                ps = psum.tile([P, H], f32, tag="bigps")
                for c in range(HC):
                    nc.tensor.transpose(
                        ps[:, c * P : (c + 1) * P], st[:, c * P : (c + 1) * P], ident
                    )
                nc.vector.tensor_copy(
                    wt.rearrange("p c (t x) -> p c t x", t=HC)[:, :, t, :], ps
                )

        # adj / s_mask: load row-major (cast i32->f32 during DMA), PE-transpose
        # into [p, b, j] layout (n = j*128 + p)
        adj4 = small.tile([BPC, N], f32)
        sm4 = small.tile([BPC, N], f32)
        nc.gpsimd.dma_start(out=adj4, in_=adjd)
        nc.gpsimd.dma_start(out=sm4, in_=smd)
        adjT = small.tile([P, BPC, NT], f32)
        smT = small.tile([P, BPC, NT], f32)
        for src4, dstT in ((adj4, adjT), (sm4, smT)):
            ps = psum.tile([P, NT, BPC], f32, tag="bigps")
            for j in range(NT):
                nc.tensor.transpose(
                    ps[:, j, :], src4[:, j * P : (j + 1) * P], ident[:BPC, :BPC]
                )
            nc.vector.tensor_copy(dstT.rearrange("p b j -> p j b"), ps)

        q_sb = small.tile([BPC, H], f32)
        nc.sync.dma_start(out=q_sb, in_=Qd)

        alphaT = small.tile([P, BPC * NT], f32)     # col = b*NT + j
        em = small.tile([P, BPC, NT], f32)          # exp(alpha)
        eT = small.tile([P, BPC, NT], f32)          # exp(alpha) * adj
        wTl = small.tile([P, BPC, NT, 2], f32r)      # stationary (w0, w1) pairs
        colsums = small.tile([P, BPC], f32)

        agg_ps = []
        for b in range(BPC):
            # ---- phase A: stream K[b], alpha[n] = K[b,n,:] . wk ----
            # DVE does all the multiplies; the reduction is split between a
            # grouped DVE reduce (tiles 0-3) and per-tile ACT accumulate
            # (tiles 4-7) to balance engine load.
            for c in range(NCH):
                base = b * NT + c * CH
                kt = kpool.tile([P, CH, H], f32)
                nc.sync.dma_start(
                    out=kt,
                    in_=Kd[b, c * CH * P : (c + 1) * CH * P, :].rearrange(
                        "(t p) h -> p t h", p=P
                    ),
                )
                scg = scratchg.tile([P, CH // 2, H], f32)
                for t in range(CH // 2):
                    nc.vector.tensor_mul(scg[:, t, :], kt[:, t, :], wkb)
                nc.vector.reduce_sum(
                    out=alphaT[:, base : base + CH // 2], in_=scg, axis=AX.X
                )
                for t in range(CH // 2, CH):
                    sc = scratch.tile([P, H], f32)
                    nc.vector.tensor_mul(sc, kt[:, t, :], wkb)
                    sc2 = scratch2.tile([P, H], f32)
                    nc.scalar.activation(
                        out=sc2,
                        in_=sc,
                        func=FT.Copy,
                        accum_out=alphaT[:, base + t : base + t + 1],
                    )

            # ---- phase A2: e = exp(alpha)*adj, unnormalized w0/w1 ----
            nc.scalar.activation(
                out=em[:, b, :],
                in_=alphaT[:, b * NT : (b + 1) * NT],
                func=FT.Exp,
            )
            nc.vector.tensor_mul(eT[:, b, :], em[:, b, :], adjT[:, b, :])
            nc.vector.reduce_sum(out=colsums[:, b : b + 1], in_=eT[:, b, :], axis=AX.X)
            nc.vector.tensor_mul(wTl[:, b, :, 0], eT[:, b, :], smT[:, b, :])
            nc.vector.tensor_sub(wTl[:, b, :, 1], eT[:, b, :], wTl[:, b, :, 0])

            # ---- phase B: stream V[b], aggregate (u0, u1) in PSUM ----
            aps = aggp.tile([2, H], f32, tag=f"agg{b}")
            agg_ps.append(aps)
            for c in range(NCH):
                vt = vpool.tile([P, CH, H], f32r)
                nc.sync.dma_start(
                    out=vt,
                    in_=Vd[b, c * CH * P : (c + 1) * CH * P, :].rearrange(
                        "(t p) h -> p t h", p=P
                    ),
                )
                for t in range(CH):
                    j = c * CH + t
                    nc.tensor.matmul(
                        aps,
                        lhsT=wTl[:, b, j, :],
                        rhs=vt[:, t, :],
                        start=(j == 0),
                        stop=(j == NT - 1),
                    )

        # ---- softmax denominators: S[b] = sum_n e  (partition-sum via PE) ----
        S_ps = outp.tile([BPC, 1], f32, tag="outps")
        nc.tensor.matmul(S_ps, lhsT=colsums, rhs=ones, start=True, stop=True)
        S_sb = small.tile([BPC, 1], f32)
        nc.scalar.copy(S_sb, S_ps)
        r = small.tile([BPC, 1], f32)
        nc.vector.reciprocal(r, S_sb)

        # ---- attn output: transpose eT -> [b, n] rows, scale by 1/S ----
        e4 = small.tile([BPC, N], f32)
        for j in range(NT):
            ps = psum.tile([BPC, P], f32, tag="bigps")
            nc.tensor.transpose(ps, eT[:, :, j], ident)
            nc.vector.tensor_copy(e4[:, j * P : (j + 1) * P], ps)
        attn_sb = small.tile([BPC, N], f32)
        nc.scalar.activation(out=attn_sb, in_=e4, func=FT.Copy, scale=r)
        nc.sync.dma_start(out=attn_out, in_=attn_sb)

        # ---- unscaled aggregates to SBUF, transpose (u0,u1)/q to [h, ...] layout ----
        u_pairs = []
        for b in range(BPC):
            up = small.tile([2, H], f32, tag=f"upair{b}")
            u_pairs.append(up)
            nc.scalar.copy(up, agg_ps[b])
        u01T = small.tile([P, HC, BPC, 2], f32r)   # [h_in_chunk, c, b, {u0,u1}]
        qT = small.tile([P, HC, BPC], f32r)
        ps_u = psum.tile([P, BPC, HC, 2], f32, tag="bigps")
        for b in range(BPC):
            for c in range(HC):
                nc.tensor.transpose(
                    ps_u[:, b, c, :], u_pairs[b][:, c * P : (c + 1) * P],
                    ident[:2, :2],
                )
        nc.vector.tensor_copy(u01T.rearrange("p c b u -> p b c u"), ps_u)
        ps_q = psum.tile([P, HC, BPC], f32, tag="bigps")
        for c in range(HC):
            nc.tensor.transpose(
                ps_q[:, c, :], q_sb[:, c * P : (c + 1) * P], ident[:BPC, :BPC]
            )
        nc.vector.tensor_copy(qT, ps_q)

        # ---- final projections ----
        # P1 = u0_raw @ Wr0^T + u1_raw @ Wr1^T   (scaled by 1/S on PSUM->SBUF copy)
        # P2 = Q @ Wri^T
        p1_ps = outp.tile([BPC, H], f32, tag="outps")
        k_i = 0
        for c in range(HC):
            for ui, wt in ((0, WTs[0]), (1, WTs[1])):
                nc.tensor.matmul(
                    p1_ps,
                    lhsT=u01T[:, c, :, ui],
                    rhs=wt[:, c, :],
                    start=(k_i == 0),
                    stop=(k_i == 2 * HC - 1),
                )
                k_i += 1
        p2_ps = qpsp.tile([BPC, H], f32, tag="qps")
        for c in range(HC):
            nc.tensor.matmul(
                p2_ps,
                lhsT=qT[:, c, :],
                rhs=WTs[2][:, c, :],
                start=(c == 0),
                stop=(c == HC - 1),
            )
        asum1 = small.tile([BPC, H], f32)
        nc.scalar.activation(out=asum1, in_=p1_ps, func=FT.Copy, scale=r)
        asum_sb = small.tile([BPC, H], f32)
        nc.vector.tensor_add(asum_sb, asum1, p2_ps)
        nc.sync.dma_start(out=asum_out, in_=asum_sb)

    nc.compile()
    return nc


_NC = None


def _get_nc():
    global _NC
    if _NC is None:
        _NC = _build()
    return _NC


def run(inputs, trace=False, tmpdir=None):
    """Run on 8 cores; returns ((attn, attn_sum), BassKernelResults)."""
    nc = _get_nc()
    Q = np.asarray(inputs["Q"], dtype=np.float32)
    K = np.asarray(inputs["K"], dtype=np.float32)
    V = np.asarray(inputs["V"], dtype=np.float32)
    adj = np.asarray(inputs["adj"], dtype=np.int32)
    s_mask = np.asarray(inputs["s_mask"], dtype=np.int32)
    w_att = np.asarray(inputs["w_att"], dtype=np.float32)
    Wr0 = np.ascontiguousarray(np.asarray(inputs["Wr0"], dtype=np.float32))
    Wr1 = np.ascontiguousarray(np.asarray(inputs["Wr1"], dtype=np.float32))
    Wri = np.ascontiguousarray(np.asarray(inputs["Wri"], dtype=np.float32))
    wk = np.ascontiguousarray(w_att[:, H:])

    in_maps = []
    for i in range(NCORES):
        sl = slice(i * BPC, (i + 1) * BPC)
        in_maps.append(
            {
                "K": np.ascontiguousarray(K[sl]),
                "V": np.ascontiguousarray(V[sl]),
                "adj": np.ascontiguousarray(adj[sl]),
                "s_mask": np.ascontiguousarray(s_mask[sl]),
                "Q": np.ascontiguousarray(Q[sl]),
                "wk": wk,
                "Wr0": Wr0,
                "Wr1": Wr1,
                "Wri": Wri,
            }
        )
    br = run_bass_kernel_spmd(
        nc, in_maps, list(range(NCORES)), trace=trace, tmpdir=tmpdir
    )
    res = br.results
    attn = np.concatenate([res[i]["attn_out"] for i in range(NCORES)], axis=0)
    asum = np.concatenate([res[i]["asum_out"] for i in range(NCORES)], axis=0)
    return (attn[:, None, :].astype(np.float32), asum.astype(np.float32)), br


def kernel(**inputs):
    (attn, asum), _ = run(inputs, trace=False)
    return attn, asum
